# revision 1
# baseline (speedup 1.0000x reference)
"""Data-parallel Trainium2 kernel for nn_DeformableTransformerDecoderLayer.

Shards the batch axis (B=16) across the 8 NeuronCores (2 per core), per the
problem's sharding hint; all gathers are batch-local so no collectives are
needed. The full decoder layer (self-attention, multi-scale deformable
attention, FFN, layernorms) runs on-device via jax/shard_map.
"""
import numpy as np
import jax
import jax.numpy as jnp
from jax.sharding import Mesh, PartitionSpec as P
from jax.experimental.shard_map import shard_map
from functools import partial

C = 256; M = 8; K = 4; L = 4; DFF = 1024; CV = C // M
SHAPES = [(100, 100), (50, 50), (25, 25), (13, 13)]
B = 16; LQ = 900
NCORES = 8

_COMPILED = {}


def _layernorm(x, g, b, eps=1e-5):
    m = x.mean(-1, keepdims=True)
    v = ((x - m) ** 2).mean(-1, keepdims=True)
    return (x - m) * jax.lax.rsqrt(v + eps) * g + b


def _bmm(a, b):
    # bf16 operands, fp32 accumulation: 1 PE pass instead of fp32's 4.
    return jnp.matmul(a.astype(jnp.bfloat16), b.astype(jnp.bfloat16),
                      preferred_element_type=jnp.float32)


def _mha(x_q, x_k, x_v, Win, bin_, Wout, bout):
    Lq, Bn, _ = x_q.shape
    q = (_bmm(x_q, Win[:, :C]) + bin_[:C]).reshape(Lq, Bn, M, CV)
    k = (_bmm(x_k, Win[:, C:2 * C]) + bin_[C:2 * C]).reshape(Lq, Bn, M, CV)
    v = (_bmm(x_v, Win[:, 2 * C:]) + bin_[2 * C:]).reshape(Lq, Bn, M, CV)
    logits = jnp.einsum('qbmd,kbmd->bmqk',
                        (q * (CV ** -0.5)).astype(jnp.bfloat16),
                        k.astype(jnp.bfloat16),
                        preferred_element_type=jnp.float32)
    # logits are small (|x| < ~4) for this layer, so an unshifted softmax is
    # numerically safe and avoids a second pass over the 900x900 matrix.
    e = jnp.exp(logits)
    a = e / e.sum(-1, keepdims=True)
    o = jnp.einsum('bmqk,kbmd->qbmd', a.astype(jnp.bfloat16),
                   v.astype(jnp.bfloat16),
                   preferred_element_type=jnp.float32).reshape(Lq, Bn, C)
    return _bmm(o, Wout) + bout


def _deform_attn(qd, ref, feats, val_w, val_b, off_w, off_b, aw_w, aw_b,
                 dout_w, dout_b):
    Bn, Lq, _ = qd.shape
    off = (_bmm(qd, off_w) + off_b).reshape(Bn, Lq, M, L, K, 2)
    awl = (_bmm(qd, aw_w) + aw_b).reshape(Bn, Lq, M, L * K)
    awe = jnp.exp(awl - awl.max(-1, keepdims=True))
    aw = (awe / awe.sum(-1, keepdims=True)).reshape(Bn, Lq, M, L, K)
    out = jnp.zeros((Bn, Lq, M, CV), qd.dtype)
    for l, feat in enumerate(feats):
        H, W = feat.shape[1], feat.shape[2]
        Hp = H + 3
        Wp = W + 3 + ((W + 3) % 2)          # even padded width
        half = Hp * Wp // 2
        v = _bmm(feat.reshape(Bn, H * W, C), val_w) + val_b
        v = v.reshape(Bn, H, W, M, CV).transpose(0, 3, 1, 2, 4)
        # zero-pad the grid (1 top/left, >=2 bottom/right); with sample coords
        # clamped to [-1, W]x[-1, H] the 2x2 bilinear footprint stays inside
        # the padded grid and out-of-range corners read zeros — exactly the
        # reference's valid-mask semantics.
        vp = jnp.pad(v, ((0, 0), (0, 0), (1, 2), (1, Wp - W - 1), (0, 0)))
        flat = vp.reshape(Bn, M, Hp * Wp * CV)
        # dual pair-table: slot s<half  -> positions (2s, 2s+1)
        #                  slot s>=half -> positions (2(s-half)+1, 2(s-half)+2)
        # so one 2-position (64-wide) gather serves any x-parity.
        A = flat.reshape(Bn, M, half, 2 * CV)
        Bv = jnp.concatenate(
            [flat[:, :, CV:], jnp.zeros((Bn, M, CV), flat.dtype)], axis=2
        ).reshape(Bn, M, half, 2 * CV)
        vd = jnp.concatenate([A, Bv], axis=2)     # (Bn, M, 2*half, 2CV)
        px = ref[:, :, None, None, 0] * W - 0.5 + off[:, :, :, l, :, 0]
        py = ref[:, :, None, None, 1] * H - 0.5 + off[:, :, :, l, :, 1]
        px = jnp.clip(px, -1.0, float(W))
        py = jnp.clip(py, -1.0, float(H))
        x0 = jnp.floor(px); y0 = jnp.floor(py)
        fx = px - x0; fy = py - y0
        xs = x0.astype(jnp.int32) + 1           # (Bn, Lq, M, K) in [0, W+1]
        ys = y0.astype(jnp.int32) + 1
        base = ys * Wp + xs
        p2 = jnp.stack([base, base + Wp], axis=-1)        # (Bn,Lq,M,K,2row)
        slot = (p2 >> 1) + (p2 & 1) * half
        slot_t = slot.transpose(0, 2, 1, 3, 4).reshape(Bn, M, Lq * K * 2)
        g = jnp.take_along_axis(vd, slot_t[..., None], axis=2)
        g = g.reshape(Bn, M, Lq, K, 2, 2, CV)   # (.., row, xcorner, CV)
        wx = jnp.stack([1.0 - fx, fx], axis=-1).transpose(0, 2, 1, 3, 4)
        wy = jnp.stack([1.0 - fy, fy], axis=-1).transpose(0, 2, 1, 3, 4)
        gx = (g * wx[:, :, :, :, None, :, None]).sum(5)   # over xcorner
        samp = (gx * wy[..., None]).sum(4)                # over row
        out = out + jnp.einsum('bqmk,bmqkc->bqmc', aw[:, :, :, l], samp)
    return _bmm(out.reshape(Bn, Lq, C), dout_w) + dout_b


def _layer_shard(query_objects, query_poses, ref_points,
                 feat0, feat1, feat2, feat3, pos0, pos1, pos2, pos3,
                 sa_in_w, sa_in_b, sa_out_w, sa_out_b,
                 n1_g, n1_b, n2_g, n2_b, n3_g, n3_b,
                 val_w, val_b, off_w, off_b, aw_w, aw_b, dout_w, dout_b,
                 ffn_w1, ffn_b1, ffn_w2, ffn_b2):
    q = query_objects + query_poses
    x = query_objects + _mha(q, q, query_objects, sa_in_w, sa_in_b,
                             sa_out_w, sa_out_b)
    x = _layernorm(x, n1_g, n1_b)
    feats = [feat0 + pos0, feat1 + pos1, feat2 + pos2, feat3 + pos3]
    qd = x.transpose(1, 0, 2)
    ref = ref_points.transpose(1, 0, 2)
    d = _deform_attn(qd, ref, feats, val_w, val_b, off_w, off_b,
                     aw_w, aw_b, dout_w, dout_b)
    x = x + d.transpose(1, 0, 2)
    x = _layernorm(x, n2_g, n2_b)
    x = x + (_bmm(jax.nn.relu(_bmm(x, ffn_w1) + ffn_b1), ffn_w2) + ffn_b2)
    x = _layernorm(x, n3_g, n3_b)
    return x


_BATCH_AXIS = {  # sharded inputs: name -> batch axis
    "query_objects": 1, "query_poses": 1, "ref_points": 1,
    "feat0": 0, "feat1": 0, "feat2": 0, "feat3": 0,
    "pos0": 0, "pos1": 0, "pos2": 0, "pos3": 0,
}

_ARG_ORDER = [
    "query_objects", "query_poses", "ref_points",
    "feat0", "feat1", "feat2", "feat3", "pos0", "pos1", "pos2", "pos3",
    "sa_in_w", "sa_in_b", "sa_out_w", "sa_out_b",
    "n1_g", "n1_b", "n2_g", "n2_b", "n3_g", "n3_b",
    "val_w", "val_b", "off_w", "off_b", "aw_w", "aw_b", "dout_w", "dout_b",
    "ffn_w1", "ffn_b1", "ffn_w2", "ffn_b2",
]


def _get_compiled():
    if "fn" in _COMPILED:
        return _COMPILED["fn"], _COMPILED["mesh"]
    devices = jax.devices()[:NCORES]
    mesh = Mesh(np.asarray(devices), ("b",))
    in_specs = tuple(
        P(*([None] * _BATCH_AXIS[n] + ["b"])) if n in _BATCH_AXIS else P()
        for n in _ARG_ORDER
    )
    fn = jax.jit(
        shard_map(_layer_shard, mesh=mesh, in_specs=in_specs,
                  out_specs=P(None, "b", None), check_rep=False)
    )
    _COMPILED["fn"] = fn
    _COMPILED["mesh"] = mesh
    return fn, mesh


def kernel(**inputs) -> np.ndarray:
    fn, _ = _get_compiled()
    args = [np.asarray(inputs[n]) for n in _ARG_ORDER]
    out = fn(*args)
    return np.asarray(jax.device_get(out)).astype(np.float32)



# revision 7
# speedup vs baseline: 1.5428x; 1.5428x over previous
"""Data-parallel Trainium2 kernel for nn_DeformableTransformerDecoderLayer.

Shards the batch axis (B=16) across the 8 NeuronCores (2 per core), per the
problem's sharding hint; all gathers are batch-local so no collectives are
needed. The full decoder layer (self-attention, multi-scale deformable
attention, FFN, layernorms) runs on-device via jax/shard_map.
"""
import numpy as np
import jax
import jax.numpy as jnp
from jax.sharding import Mesh, PartitionSpec as P
from jax.experimental.shard_map import shard_map
from functools import partial

C = 256; M = 8; K = 4; L = 4; DFF = 1024; CV = C // M
SHAPES = [(100, 100), (50, 50), (25, 25), (13, 13)]
B = 16; LQ = 900
NCORES = 8

_COMPILED = {}


def _layernorm(x, g, b, eps=1e-5):
    m = x.mean(-1, keepdims=True)
    v = ((x - m) ** 2).mean(-1, keepdims=True)
    return (x - m) * jax.lax.rsqrt(v + eps) * g + b


def _bmm(a, b):
    # bf16 operands, fp32 accumulation: 1 PE pass instead of fp32's 4.
    return jnp.matmul(a.astype(jnp.bfloat16), b.astype(jnp.bfloat16),
                      preferred_element_type=jnp.float32)


def _mha(x_q, x_k, x_v, Win, bin_, Wout, bout):
    Lq, Bn, _ = x_q.shape
    xq2 = x_q.reshape(Lq * Bn, C)
    xv2 = x_v.reshape(Lq * Bn, C)
    q = (_bmm(xq2, Win[:, :C]) + bin_[:C]).reshape(Lq, Bn, M, CV)
    k = (_bmm(xq2, Win[:, C:2 * C]) + bin_[C:2 * C]).reshape(Lq, Bn, M, CV)
    v = (_bmm(xv2, Win[:, 2 * C:]) + bin_[2 * C:]).reshape(Lq, Bn, M, CV)
    logits = jnp.einsum('qbmd,kbmd->bmqk',
                        (q * (CV ** -0.5)).astype(jnp.bfloat16),
                        k.astype(jnp.bfloat16),
                        preferred_element_type=jnp.float32)
    # logits are small (|x| < ~4) for this layer, so an unshifted softmax is
    # numerically safe and avoids a second pass over the 900x900 matrix.
    e = jnp.exp(logits)
    a = e / e.sum(-1, keepdims=True)
    o = jnp.einsum('bmqk,kbmd->qbmd', a.astype(jnp.bfloat16),
                   v.astype(jnp.bfloat16),
                   preferred_element_type=jnp.float32).reshape(Lq * Bn, C)
    return (_bmm(o, Wout) + bout).reshape(Lq, Bn, C)


def _deform_attn(qd, ref, feats, val_w, val_b, off_w, off_b, aw_w, aw_b,
                 dout_w, dout_b):
    Bn, Lq, _ = qd.shape
    qd2 = qd.reshape(Bn * Lq, C)
    off = (_bmm(qd2, off_w) + off_b).reshape(Bn, Lq, M, L, K, 2)
    awl = (_bmm(qd2, aw_w) + aw_b).reshape(Bn, Lq, M, L * K)
    awe = jnp.exp(awl - awl.max(-1, keepdims=True))
    aw = (awe / awe.sum(-1, keepdims=True)).reshape(Bn, Lq, M, L, K)
    # one 2D val projection over all levels (XLA-neuron handles batched/3D
    # matmuls poorly; a single flat [P,256]@[256,256] hits the PE roofline)
    flat = jnp.concatenate([f.reshape(Bn, -1, C) for f in feats], axis=1)
    vall = _bmm(flat.reshape(-1, C), val_w) + val_b
    vall = vall.reshape(Bn, -1, C)
    lvl_off = [0]
    for f in feats:
        lvl_off.append(lvl_off[-1] + f.shape[1] * f.shape[2])
    out = jnp.zeros((Bn, Lq, M, CV), qd.dtype)
    for l, feat in enumerate(feats):
        H, W = feat.shape[1], feat.shape[2]
        Hp = H + 3 + ((H + 3) % 2)          # even padded height
        Wp = W + 3 + ((W + 3) % 2)          # even padded width
        HB, WB = Hp // 2, Wp // 2
        v = vall[:, lvl_off[l]:lvl_off[l + 1]]
        v = v.reshape(Bn, H, W, M, CV).transpose(0, 3, 1, 2, 4)
        # zero-pad the grid (1 top/left, rest bottom/right, +1 for the
        # shifted-window views); sample coords are clamped to [-1,W]x[-1,H]
        # so the 2x2 bilinear footprint stays inside the padded grid and
        # out-of-range corners carry weight 0 (reference's valid mask).
        vp = jnp.pad(v, ((0, 0), (0, 0), (1, Hp - H), (1, Wp - W), (0, 0)))
        # quad-parity 2x2-block table: one 128-wide gather row holds all 4
        # bilinear corners of a sample point. Variant (dy,dx) covers
        # footprints whose top-left grid coord has that parity. This halves
        # the index count vs. per-row pair gathers — the XLA-neuron gather
        # costs ~per-index, independent of row width.
        Tq = jnp.concatenate([
            vp[:, :, dy:dy + 2 * HB, dx:dx + 2 * WB, :]
            .reshape(Bn, M, HB, 2, WB, 2, CV)
            .transpose(0, 1, 2, 4, 3, 5, 6)
            .reshape(Bn, M, HB * WB, 4 * CV)
            for dy in (0, 1) for dx in (0, 1)], axis=2)
        px = ref[:, :, None, None, 0] * W - 0.5 + off[:, :, :, l, :, 0]
        py = ref[:, :, None, None, 1] * H - 0.5 + off[:, :, :, l, :, 1]
        px = jnp.clip(px, -1.0, float(W))
        py = jnp.clip(py, -1.0, float(H))
        x0 = jnp.floor(px); y0 = jnp.floor(py)
        fx = px - x0; fy = py - y0
        xs = x0.astype(jnp.int32) + 1           # (Bn, Lq, M, K) in [0, W+1]
        ys = y0.astype(jnp.int32) + 1
        vi = (ys & 1) * 2 + (xs & 1)
        row = (vi * HB + (ys >> 1)) * WB + (xs >> 1)
        row_t = row.transpose(0, 2, 1, 3).reshape(Bn, M, Lq * K)
        g = jnp.take_along_axis(Tq, row_t[..., None], axis=2)
        g = g.reshape(Bn, M, Lq, K, 2, 2, CV)   # (.., ycorner, xcorner, CV)
        wx = jnp.stack([1.0 - fx, fx], axis=-1).transpose(0, 2, 1, 3, 4)
        wy = jnp.stack([1.0 - fy, fy], axis=-1).transpose(0, 2, 1, 3, 4)
        gx = (g * wx[:, :, :, :, None, :, None]).sum(5)   # over xcorner
        samp = (gx * wy[..., None]).sum(4)                # over ycorner
        out = out + jnp.einsum('bqmk,bmqkc->bqmc', aw[:, :, :, l], samp)
    return (_bmm(out.reshape(Bn * Lq, C), dout_w) + dout_b).reshape(Bn, Lq, C)


def _layer_shard(query_objects, query_poses, ref_points,
                 feat0, feat1, feat2, feat3, pos0, pos1, pos2, pos3,
                 sa_in_w, sa_in_b, sa_out_w, sa_out_b,
                 n1_g, n1_b, n2_g, n2_b, n3_g, n3_b,
                 val_w, val_b, off_w, off_b, aw_w, aw_b, dout_w, dout_b,
                 ffn_w1, ffn_b1, ffn_w2, ffn_b2):
    q = query_objects + query_poses
    x = query_objects + _mha(q, q, query_objects, sa_in_w, sa_in_b,
                             sa_out_w, sa_out_b)
    x = _layernorm(x, n1_g, n1_b)
    feats = [feat0 + pos0, feat1 + pos1, feat2 + pos2, feat3 + pos3]
    qd = x.transpose(1, 0, 2)
    ref = ref_points.transpose(1, 0, 2)
    d = _deform_attn(qd, ref, feats, val_w, val_b, off_w, off_b,
                     aw_w, aw_b, dout_w, dout_b)
    x = x + d.transpose(1, 0, 2)
    x = _layernorm(x, n2_g, n2_b)
    x2 = x.reshape(-1, C)
    x2 = x2 + (_bmm(jax.nn.relu(_bmm(x2, ffn_w1) + ffn_b1), ffn_w2) + ffn_b2)
    x = _layernorm(x2.reshape(x.shape), n3_g, n3_b)
    return x


_BATCH_AXIS = {  # sharded inputs: name -> batch axis
    "query_objects": 1, "query_poses": 1, "ref_points": 1,
    "feat0": 0, "feat1": 0, "feat2": 0, "feat3": 0,
    "pos0": 0, "pos1": 0, "pos2": 0, "pos3": 0,
}

_ARG_ORDER = [
    "query_objects", "query_poses", "ref_points",
    "feat0", "feat1", "feat2", "feat3", "pos0", "pos1", "pos2", "pos3",
    "sa_in_w", "sa_in_b", "sa_out_w", "sa_out_b",
    "n1_g", "n1_b", "n2_g", "n2_b", "n3_g", "n3_b",
    "val_w", "val_b", "off_w", "off_b", "aw_w", "aw_b", "dout_w", "dout_b",
    "ffn_w1", "ffn_b1", "ffn_w2", "ffn_b2",
]


def _get_compiled():
    if "fn" in _COMPILED:
        return _COMPILED["fn"], _COMPILED["mesh"]
    devices = jax.devices()[:NCORES]
    mesh = Mesh(np.asarray(devices), ("b",))
    in_specs = tuple(
        P(*([None] * _BATCH_AXIS[n] + ["b"])) if n in _BATCH_AXIS else P()
        for n in _ARG_ORDER
    )
    fn = jax.jit(
        shard_map(_layer_shard, mesh=mesh, in_specs=in_specs,
                  out_specs=P(None, "b", None), check_rep=False)
    )
    _COMPILED["fn"] = fn
    _COMPILED["mesh"] = mesh
    return fn, mesh


def kernel(**inputs) -> np.ndarray:
    fn, _ = _get_compiled()
    args = [np.asarray(inputs[n]) for n in _ARG_ORDER]
    out = fn(*args)
    return np.asarray(jax.device_get(out)).astype(np.float32)



# revision 9
# speedup vs baseline: 1.6088x; 1.0428x over previous
"""Data-parallel Trainium2 kernel for nn_DeformableTransformerDecoderLayer.

Shards the batch axis (B=16) across the 8 NeuronCores (2 per core), per the
problem's sharding hint; all gathers are batch-local so no collectives are
needed. The full decoder layer (self-attention, multi-scale deformable
attention, FFN, layernorms) runs on-device via jax/shard_map.
"""
import numpy as np
import jax
import jax.numpy as jnp
from jax.sharding import Mesh, PartitionSpec as P
from jax.experimental.shard_map import shard_map
from functools import partial

C = 256; M = 8; K = 4; L = 4; DFF = 1024; CV = C // M
SHAPES = [(100, 100), (50, 50), (25, 25), (13, 13)]
B = 16; LQ = 900
NCORES = 8

_COMPILED = {}


def _layernorm(x, g, b, eps=1e-5):
    m = x.mean(-1, keepdims=True)
    v = ((x - m) ** 2).mean(-1, keepdims=True)
    return (x - m) * jax.lax.rsqrt(v + eps) * g + b


def _bmm(a, b):
    # bf16 operands, fp32 accumulation: 1 PE pass instead of fp32's 4.
    return jnp.matmul(a.astype(jnp.bfloat16), b.astype(jnp.bfloat16),
                      preferred_element_type=jnp.float32)


def _mha(x_q, x_k, x_v, Win, bin_, Wout, bout):
    Lq, Bn, _ = x_q.shape
    xq2 = x_q.reshape(Lq * Bn, C)
    xv2 = x_v.reshape(Lq * Bn, C)
    q = (_bmm(xq2, Win[:, :C]) + bin_[:C]).reshape(Lq, Bn, M, CV)
    k = (_bmm(xq2, Win[:, C:2 * C]) + bin_[C:2 * C]).reshape(Lq, Bn, M, CV)
    v = (_bmm(xv2, Win[:, 2 * C:]) + bin_[2 * C:]).reshape(Lq, Bn, M, CV)
    logits = jnp.einsum('qbmd,kbmd->bmqk',
                        (q * (CV ** -0.5)).astype(jnp.bfloat16),
                        k.astype(jnp.bfloat16),
                        preferred_element_type=jnp.float32)
    # logits are small (|x| < ~4) for this layer, so an unshifted softmax is
    # numerically safe and avoids a second pass over the 900x900 matrix.
    e = jnp.exp(logits)
    a = e / e.sum(-1, keepdims=True)
    o = jnp.einsum('bmqk,kbmd->qbmd', a.astype(jnp.bfloat16),
                   v.astype(jnp.bfloat16),
                   preferred_element_type=jnp.float32).reshape(Lq * Bn, C)
    return (_bmm(o, Wout) + bout).reshape(Lq, Bn, C)


def _deform_attn(qd, ref, feats, val_w, val_b, off_w, off_b, aw_w, aw_b,
                 dout_w, dout_b):
    Bn, Lq, _ = qd.shape
    qd2 = qd.reshape(Bn * Lq, C)
    off = (_bmm(qd2, off_w) + off_b).reshape(Bn, Lq, M, L, K, 2)
    awl = (_bmm(qd2, aw_w) + aw_b).reshape(Bn, Lq, M, L * K)
    awe = jnp.exp(awl - awl.max(-1, keepdims=True))
    aw = (awe / awe.sum(-1, keepdims=True)).reshape(Bn, Lq, M, L, K)
    # one 2D val projection over all levels (XLA-neuron handles batched/3D
    # matmuls poorly; a single flat [P,256]@[256,256] hits the PE roofline)
    flat = jnp.concatenate([f.reshape(Bn, -1, C) for f in feats], axis=1)
    vall = _bmm(flat.reshape(-1, C), val_w) + val_b
    vall = vall.reshape(Bn, -1, C)
    lvl_off = [0]
    for f in feats:
        lvl_off.append(lvl_off[-1] + f.shape[1] * f.shape[2])
    out = jnp.zeros((Bn, Lq, M, CV), qd.dtype)
    for l, feat in enumerate(feats):
        H, W = feat.shape[1], feat.shape[2]
        Hp = H + 3 + ((H + 3) % 2)          # even padded height
        Wp = W + 3 + ((W + 3) % 2)          # even padded width
        HB, WB = Hp // 2, Wp // 2
        v = vall[:, lvl_off[l]:lvl_off[l + 1]]
        v = v.reshape(Bn, H, W, M, CV).transpose(0, 3, 1, 2, 4)
        # zero-pad the grid (1 top/left, rest bottom/right, +1 for the
        # shifted-window views); sample coords are clamped to [-1,W]x[-1,H]
        # so the 2x2 bilinear footprint stays inside the padded grid and
        # out-of-range corners carry weight 0 (reference's valid mask).
        vp = jnp.pad(v, ((0, 0), (0, 0), (1, Hp - H), (1, Wp - W), (0, 0)))
        # quad-parity 2x2-block table: one 128-wide gather row holds all 4
        # bilinear corners of a sample point. Variant (dy,dx) covers
        # footprints whose top-left grid coord has that parity. This halves
        # the index count vs. per-row pair gathers — the XLA-neuron gather
        # costs ~per-index, independent of row width.
        Tq = jnp.concatenate([
            vp[:, :, dy:dy + 2 * HB, dx:dx + 2 * WB, :]
            .reshape(Bn, M, HB, 2, WB, 2, CV)
            .transpose(0, 1, 2, 4, 3, 5, 6)
            .reshape(Bn, M, HB * WB, 4 * CV)
            for dy in (0, 1) for dx in (0, 1)], axis=2).astype(jnp.bfloat16)
        px = ref[:, :, None, None, 0] * W - 0.5 + off[:, :, :, l, :, 0]
        py = ref[:, :, None, None, 1] * H - 0.5 + off[:, :, :, l, :, 1]
        px = jnp.clip(px, -1.0, float(W))
        py = jnp.clip(py, -1.0, float(H))
        x0 = jnp.floor(px); y0 = jnp.floor(py)
        fx = px - x0; fy = py - y0
        xs = x0.astype(jnp.int32) + 1           # (Bn, Lq, M, K) in [0, W+1]
        ys = y0.astype(jnp.int32) + 1
        vi = (ys & 1) * 2 + (xs & 1)
        row = (vi * HB + (ys >> 1)) * WB + (xs >> 1)
        row_t = row.transpose(0, 2, 1, 3).reshape(Bn, M, Lq * K)
        g = jnp.take_along_axis(Tq, row_t[..., None], axis=2)
        g = g.reshape(Bn, M, Lq, K, 2, 2, CV).astype(jnp.float32)
        wx = jnp.stack([1.0 - fx, fx], axis=-1).transpose(0, 2, 1, 3, 4)
        wy = jnp.stack([1.0 - fy, fy], axis=-1).transpose(0, 2, 1, 3, 4)
        gx = (g * wx[:, :, :, :, None, :, None]).sum(5)   # over xcorner
        samp = (gx * wy[..., None]).sum(4)                # over ycorner
        out = out + jnp.einsum('bqmk,bmqkc->bqmc', aw[:, :, :, l], samp)
    return (_bmm(out.reshape(Bn * Lq, C), dout_w) + dout_b).reshape(Bn, Lq, C)


def _layer_shard(query_objects, query_poses, ref_points,
                 feat0, feat1, feat2, feat3, pos0, pos1, pos2, pos3,
                 sa_in_w, sa_in_b, sa_out_w, sa_out_b,
                 n1_g, n1_b, n2_g, n2_b, n3_g, n3_b,
                 val_w, val_b, off_w, off_b, aw_w, aw_b, dout_w, dout_b,
                 ffn_w1, ffn_b1, ffn_w2, ffn_b2):
    q = query_objects + query_poses
    x = query_objects + _mha(q, q, query_objects, sa_in_w, sa_in_b,
                             sa_out_w, sa_out_b)
    x = _layernorm(x, n1_g, n1_b)
    feats = [feat0 + pos0, feat1 + pos1, feat2 + pos2, feat3 + pos3]
    qd = x.transpose(1, 0, 2)
    ref = ref_points.transpose(1, 0, 2)
    d = _deform_attn(qd, ref, feats, val_w, val_b, off_w, off_b,
                     aw_w, aw_b, dout_w, dout_b)
    x = x + d.transpose(1, 0, 2)
    x = _layernorm(x, n2_g, n2_b)
    x2 = x.reshape(-1, C)
    x2 = x2 + (_bmm(jax.nn.relu(_bmm(x2, ffn_w1) + ffn_b1), ffn_w2) + ffn_b2)
    x = _layernorm(x2.reshape(x.shape), n3_g, n3_b)
    return x


_BATCH_AXIS = {  # sharded inputs: name -> batch axis
    "query_objects": 1, "query_poses": 1, "ref_points": 1,
    "feat0": 0, "feat1": 0, "feat2": 0, "feat3": 0,
    "pos0": 0, "pos1": 0, "pos2": 0, "pos3": 0,
}

_ARG_ORDER = [
    "query_objects", "query_poses", "ref_points",
    "feat0", "feat1", "feat2", "feat3", "pos0", "pos1", "pos2", "pos3",
    "sa_in_w", "sa_in_b", "sa_out_w", "sa_out_b",
    "n1_g", "n1_b", "n2_g", "n2_b", "n3_g", "n3_b",
    "val_w", "val_b", "off_w", "off_b", "aw_w", "aw_b", "dout_w", "dout_b",
    "ffn_w1", "ffn_b1", "ffn_w2", "ffn_b2",
]


def _get_compiled():
    if "fn" in _COMPILED:
        return _COMPILED["fn"], _COMPILED["mesh"]
    devices = jax.devices()[:NCORES]
    mesh = Mesh(np.asarray(devices), ("b",))
    in_specs = tuple(
        P(*([None] * _BATCH_AXIS[n] + ["b"])) if n in _BATCH_AXIS else P()
        for n in _ARG_ORDER
    )
    fn = jax.jit(
        shard_map(_layer_shard, mesh=mesh, in_specs=in_specs,
                  out_specs=P(None, "b", None), check_rep=False)
    )
    _COMPILED["fn"] = fn
    _COMPILED["mesh"] = mesh
    return fn, mesh


def kernel(**inputs) -> np.ndarray:
    fn, _ = _get_compiled()
    args = [np.asarray(inputs[n]) for n in _ARG_ORDER]
    out = fn(*args)
    return np.asarray(jax.device_get(out)).astype(np.float32)



# revision 10
# speedup vs baseline: 1.8867x; 1.1728x over previous
"""Data-parallel Trainium2 kernel for nn_DeformableTransformerDecoderLayer.

Shards the batch axis (B=16) across the 8 NeuronCores (2 per core), per the
problem's sharding hint; all gathers are batch-local so no collectives are
needed. The full decoder layer (self-attention, multi-scale deformable
attention, FFN, layernorms) runs on-device via jax/shard_map.
"""
import numpy as np
import jax
import jax.numpy as jnp
from jax.sharding import Mesh, PartitionSpec as P
from jax.experimental.shard_map import shard_map
from functools import partial

C = 256; M = 8; K = 4; L = 4; DFF = 1024; CV = C // M
SHAPES = [(100, 100), (50, 50), (25, 25), (13, 13)]
B = 16; LQ = 900
NCORES = 8

_COMPILED = {}


def _layernorm(x, g, b, eps=1e-5):
    m = x.mean(-1, keepdims=True)
    v = ((x - m) ** 2).mean(-1, keepdims=True)
    return (x - m) * jax.lax.rsqrt(v + eps) * g + b


def _bmm(a, b):
    # bf16 operands, fp32 accumulation: 1 PE pass instead of fp32's 4.
    return jnp.matmul(a.astype(jnp.bfloat16), b.astype(jnp.bfloat16),
                      preferred_element_type=jnp.float32)


def _mha(x_q, x_k, x_v, Win, bin_, Wout, bout):
    Lq, Bn, _ = x_q.shape
    xq2 = x_q.reshape(Lq * Bn, C)
    xv2 = x_v.reshape(Lq * Bn, C)
    q = (_bmm(xq2, Win[:, :C]) + bin_[:C]).reshape(Lq, Bn, M, CV)
    k = (_bmm(xq2, Win[:, C:2 * C]) + bin_[C:2 * C]).reshape(Lq, Bn, M, CV)
    v = (_bmm(xv2, Win[:, 2 * C:]) + bin_[2 * C:]).reshape(Lq, Bn, M, CV)
    logits = jnp.einsum('qbmd,kbmd->bmqk',
                        (q * (CV ** -0.5)).astype(jnp.bfloat16),
                        k.astype(jnp.bfloat16),
                        preferred_element_type=jnp.float32)
    # logits are small (|x| < ~4) for this layer, so an unshifted softmax is
    # numerically safe and avoids a second pass over the 900x900 matrix.
    e = jnp.exp(logits)
    a = e / e.sum(-1, keepdims=True)
    o = jnp.einsum('bmqk,kbmd->qbmd', a.astype(jnp.bfloat16),
                   v.astype(jnp.bfloat16),
                   preferred_element_type=jnp.float32).reshape(Lq * Bn, C)
    return (_bmm(o, Wout) + bout).reshape(Lq, Bn, C)


def _deform_attn(qd, ref, feats, val_w, val_b, off_w, off_b, aw_w, aw_b,
                 dout_w, dout_b):
    Bn, Lq, _ = qd.shape
    qd2 = qd.reshape(Bn * Lq, C)
    off = (_bmm(qd2, off_w) + off_b).reshape(Bn, Lq, M, L, K, 2)
    awl = (_bmm(qd2, aw_w) + aw_b).reshape(Bn, Lq, M, L * K)
    awe = jnp.exp(awl - awl.max(-1, keepdims=True))
    aw = (awe / awe.sum(-1, keepdims=True)).reshape(Bn, Lq, M, L, K)
    # one 2D val projection over all levels (XLA-neuron handles batched/3D
    # matmuls poorly; a single flat [P,256]@[256,256] hits the PE roofline)
    flat = jnp.concatenate([f.reshape(Bn, -1, C) for f in feats], axis=1)
    vall = _bmm(flat.reshape(-1, C), val_w) + val_b
    vall = vall.reshape(Bn, -1, C)
    lvl_off = [0]
    for f in feats:
        lvl_off.append(lvl_off[-1] + f.shape[1] * f.shape[2])
    out = jnp.zeros((Bn, Lq, M, CV), qd.dtype)
    for l, feat in enumerate(feats):
        H, W = feat.shape[1], feat.shape[2]
        Hp = H + 3 + ((H + 3) % 2)          # even padded height
        Wp = W + 3 + ((W + 3) % 2)          # even padded width
        HB, WB = Hp // 2, Wp // 2
        v = vall[:, lvl_off[l]:lvl_off[l + 1]]
        v = v.reshape(Bn, H, W, M, CV).transpose(0, 3, 1, 2, 4)
        # zero-pad the grid (1 top/left, rest bottom/right, +1 for the
        # shifted-window views); sample coords are clamped to [-1,W]x[-1,H]
        # so the 2x2 bilinear footprint stays inside the padded grid and
        # out-of-range corners carry weight 0 (reference's valid mask).
        vp = jnp.pad(v, ((0, 0), (0, 0), (1, Hp - H), (1, Wp - W), (0, 0)))
        # quad-parity 2x2-block table: one 128-wide gather row holds all 4
        # bilinear corners of a sample point. Variant (dy,dx) covers
        # footprints whose top-left grid coord has that parity. This halves
        # the index count vs. per-row pair gathers — the XLA-neuron gather
        # costs ~per-index, independent of row width.
        Tq = jnp.concatenate([
            vp[:, :, dy:dy + 2 * HB, dx:dx + 2 * WB, :]
            .reshape(Bn, M, HB, 2, WB, 2, CV)
            .transpose(0, 1, 2, 4, 3, 5, 6)
            .reshape(Bn, M, HB * WB, 4 * CV)
            for dy in (0, 1) for dx in (0, 1)], axis=2).astype(jnp.bfloat16)
        px = ref[:, :, None, None, 0] * W - 0.5 + off[:, :, :, l, :, 0]
        py = ref[:, :, None, None, 1] * H - 0.5 + off[:, :, :, l, :, 1]
        px = jnp.clip(px, -1.0, float(W))
        py = jnp.clip(py, -1.0, float(H))
        x0 = jnp.floor(px); y0 = jnp.floor(py)
        fx = px - x0; fy = py - y0
        xs = x0.astype(jnp.int32) + 1           # (Bn, Lq, M, K) in [0, W+1]
        ys = y0.astype(jnp.int32) + 1
        vi = (ys & 1) * 2 + (xs & 1)
        row = (vi * HB + (ys >> 1)) * WB + (xs >> 1)
        row_t = row.transpose(0, 2, 1, 3).reshape(Bn, M, Lq * K)
        g = jnp.take_along_axis(Tq, row_t[..., None], axis=2,
                                mode='promise_in_bounds')
        g = g.reshape(Bn, M, Lq, K, 2, 2, CV)
        # fold attention weight and both bilinear factors into one small
        # per-point weight, then a single fused contraction over the big
        # gathered tensor (one pass instead of four).
        wx = jnp.stack([1.0 - fx, fx], axis=-1)           # (B, Lq, M, K, 2)
        wy = jnp.stack([1.0 - fy, fy], axis=-1)
        w4 = (aw[:, :, :, l, :, None, None] * wy[..., :, None]
              * wx[..., None, :]).astype(jnp.bfloat16)    # (B,Lq,M,K,2,2)
        out = out + jnp.einsum('bqmkyx,bmqkyxc->bqmc', w4, g,
                               preferred_element_type=jnp.float32)
    return (_bmm(out.reshape(Bn * Lq, C), dout_w) + dout_b).reshape(Bn, Lq, C)


def _layer_shard(query_objects, query_poses, ref_points,
                 feat0, feat1, feat2, feat3, pos0, pos1, pos2, pos3,
                 sa_in_w, sa_in_b, sa_out_w, sa_out_b,
                 n1_g, n1_b, n2_g, n2_b, n3_g, n3_b,
                 val_w, val_b, off_w, off_b, aw_w, aw_b, dout_w, dout_b,
                 ffn_w1, ffn_b1, ffn_w2, ffn_b2):
    q = query_objects + query_poses
    x = query_objects + _mha(q, q, query_objects, sa_in_w, sa_in_b,
                             sa_out_w, sa_out_b)
    x = _layernorm(x, n1_g, n1_b)
    feats = [feat0 + pos0, feat1 + pos1, feat2 + pos2, feat3 + pos3]
    qd = x.transpose(1, 0, 2)
    ref = ref_points.transpose(1, 0, 2)
    d = _deform_attn(qd, ref, feats, val_w, val_b, off_w, off_b,
                     aw_w, aw_b, dout_w, dout_b)
    x = x + d.transpose(1, 0, 2)
    x = _layernorm(x, n2_g, n2_b)
    x2 = x.reshape(-1, C)
    x2 = x2 + (_bmm(jax.nn.relu(_bmm(x2, ffn_w1) + ffn_b1), ffn_w2) + ffn_b2)
    x = _layernorm(x2.reshape(x.shape), n3_g, n3_b)
    return x


_BATCH_AXIS = {  # sharded inputs: name -> batch axis
    "query_objects": 1, "query_poses": 1, "ref_points": 1,
    "feat0": 0, "feat1": 0, "feat2": 0, "feat3": 0,
    "pos0": 0, "pos1": 0, "pos2": 0, "pos3": 0,
}

_ARG_ORDER = [
    "query_objects", "query_poses", "ref_points",
    "feat0", "feat1", "feat2", "feat3", "pos0", "pos1", "pos2", "pos3",
    "sa_in_w", "sa_in_b", "sa_out_w", "sa_out_b",
    "n1_g", "n1_b", "n2_g", "n2_b", "n3_g", "n3_b",
    "val_w", "val_b", "off_w", "off_b", "aw_w", "aw_b", "dout_w", "dout_b",
    "ffn_w1", "ffn_b1", "ffn_w2", "ffn_b2",
]


def _get_compiled():
    if "fn" in _COMPILED:
        return _COMPILED["fn"], _COMPILED["mesh"]
    devices = jax.devices()[:NCORES]
    mesh = Mesh(np.asarray(devices), ("b",))
    in_specs = tuple(
        P(*([None] * _BATCH_AXIS[n] + ["b"])) if n in _BATCH_AXIS else P()
        for n in _ARG_ORDER
    )
    fn = jax.jit(
        shard_map(_layer_shard, mesh=mesh, in_specs=in_specs,
                  out_specs=P(None, "b", None), check_rep=False)
    )
    _COMPILED["fn"] = fn
    _COMPILED["mesh"] = mesh
    return fn, mesh


def kernel(**inputs) -> np.ndarray:
    fn, _ = _get_compiled()
    args = [np.asarray(inputs[n]) for n in _ARG_ORDER]
    out = fn(*args)
    return np.asarray(jax.device_get(out)).astype(np.float32)

